# revision 10
# baseline (speedup 1.0000x reference)
"""2D Haar DWT (periodized, 2-tap orthogonal filter bank) on Trainium2.

Reference computes, per batch & channel, y = A @ X @ A^T with A the
2-sparse Haar analysis matrix, then stacks the LL/LH/HL/HH quadrants on
the channel axis.  Because every row of A has exactly two taps
(lowpass p = A[0,0] twice; highpass q = A[H,0], -q), the whole thing is
an elementwise 2x2 butterfly:

    S = E + O     (row pairs: even rows E, odd rows O)
    D = E - O
    LL = p*p*(S_e + S_o)   LH = p*q*(D_e + D_o)
    HL = p*q*(S_e - S_o)   HH = q*q*(D_e - D_o)

which is memory-bound: 16 MiB in + 16 MiB out per core.

Sharding: data-parallel over batch.  Core b gets x[b] (512,512,16) and
produces out[b] (256,256,64).  The two filter taps are read from A on
the host and baked into the program as immediates, so A is never DMA'd.

Engine split per column chunk (DVE is the scarce engine):
  DVE:    S = E+O, and the four quadrant butterflies
  GpSimd: D = E-O (contiguous 2-input op, ~2x DVE cost but POOL is idle)
  ACT:    output scaling (one fused op when p == q, else per-quadrant)
  Sync:   HWDGE DMA
"""

import numpy as np

B, N, C = 8, 512, 16
H = N // 2
P = 128                 # SBUF partitions
COL_CHUNK = 128         # input columns per chunk
FE = COL_CHUNK * C      # free elems of an E/O/S/D tile  (4096)
FV = (COL_CHUNK // 2) * 4 * C  # free elems of a V (output) tile (8192)

_PROGRAM_CACHE = {}


def _build_program(p: float, q: float):
    import concourse.bacc as bacc
    import concourse.mybir as mybir
    from concourse.tile import TileContext

    f32 = mybir.dt.float32
    nc = bacc.Bacc("TRN2", target_bir_lowering=False)

    x = nc.dram_tensor("x", [N, N, C], f32, kind="ExternalInput")
    out = nc.dram_tensor("out", [H, H, 4 * C], f32, kind="ExternalOutput")

    # [2, 256, 8192]: even/odd input rows, flattened (col, chan) free dim
    xr = x[:, :, :].rearrange("(k two) w c -> two k (w c)", two=2)
    # [256, 16384]: output rows, flattened (col, chan) free dim
    of = out[:, :, :].rearrange("k m c -> k (m c)")

    pp, pq, qq = p * p, p * q, q * q
    uniform_scale = abs(p - q) < 1e-12

    # Column chunking: 64-col chunks at the very start (shorter fill: the
    # first butterfly only waits on 2x0.5 MiB of DMA) and at the very end
    # (shorter drain for the last compute->scale->store chain); 128-col
    # chunks in the middle.
    chunks_by_rt = [
        [(0, 64), (64, 64), (128, 128), (256, 128), (384, 128)],
        [(0, 128), (128, 128), (256, 128), (384, 64), (448, 64)],
    ]

    with TileContext(nc) as tc:
        with tc.tile_pool(name="pool", bufs=4) as pool:
            for rt in range(H // P):              # 2 row tiles of 128 row-pairs
                r0 = rt * P
                for c0, clen in chunks_by_rt[rt]:
                    fe = clen * C
                    fv = (clen // 2) * 4 * C
                    f0 = c0 * C
                    e = pool.tile([P, fe], f32, tag="e")
                    o = pool.tile([P, fe], f32, tag="o")
                    nc.sync.dma_start(out=e, in_=xr[0, r0 : r0 + P, f0 : f0 + fe])
                    nc.sync.dma_start(out=o, in_=xr[1, r0 : r0 + P, f0 : f0 + fe])

                    s = pool.tile([P, fe], f32, bufs=2, tag="s")
                    d = pool.tile([P, fe], f32, bufs=2, tag="d")
                    nc.vector.tensor_add(out=s, in0=e, in1=o)
                    nc.vector.tensor_sub(out=d, in0=e, in1=o)

                    s4 = s.rearrange("p (m two c) -> p m two c", two=2, c=C)
                    d4 = d.rearrange("p (m two c) -> p m two c", two=2, c=C)
                    v = pool.tile([P, fv], f32, tag="v")
                    v4 = v.rearrange("p (m q c) -> p m q c", q=4, c=C)

                    nc.vector.tensor_add(out=v4[:, :, 0, :], in0=s4[:, :, 0, :], in1=s4[:, :, 1, :])
                    nc.vector.tensor_add(out=v4[:, :, 1, :], in0=d4[:, :, 0, :], in1=d4[:, :, 1, :])
                    nc.vector.tensor_sub(out=v4[:, :, 2, :], in0=s4[:, :, 0, :], in1=s4[:, :, 1, :])
                    nc.vector.tensor_sub(out=v4[:, :, 3, :], in0=d4[:, :, 0, :], in1=d4[:, :, 1, :])

                    # Scale split by quadrant pair so each half only waits on
                    # its two butterflies (overlaps ACT with DVE stage 2).
                    vq = v.rearrange("p (m qc) -> p m qc", qc=4 * C)
                    if uniform_scale:
                        nc.scalar.mul(vq[:, :, 0 : 2 * C], vq[:, :, 0 : 2 * C], pp)
                        nc.scalar.mul(vq[:, :, 2 * C : 4 * C], vq[:, :, 2 * C : 4 * C], pq)
                    else:
                        nc.scalar.mul(v4[:, :, 0, :], v4[:, :, 0, :], pp)
                        nc.scalar.mul(v4[:, :, 1, :], v4[:, :, 1, :], pq)
                        nc.scalar.mul(v4[:, :, 2, :], v4[:, :, 2, :], pq)
                        nc.scalar.mul(v4[:, :, 3, :], v4[:, :, 3, :], qq)

                    g0 = (c0 // 2) * 4 * C
                    # out-DMA on the scalar engine (HWDGE): it directly
                    # follows its only producer (the ACT scale) in that
                    # engine's stream, and keeps the Sync engine's in-order
                    # stream free for input prefetch.
                    nc.scalar.dma_start(out=of[r0 : r0 + P, g0 : g0 + fv], in_=v)

    nc.finalize()
    return nc


LAST_RESULTS = None  # BassKernelResults of the most recent run (for test harness)


def kernel(x: np.ndarray, A: np.ndarray) -> np.ndarray:
    from concourse.bass_utils import run_bass_kernel_spmd

    global LAST_RESULTS

    x = np.ascontiguousarray(np.asarray(x, dtype=np.float32))
    A = np.asarray(A, dtype=np.float32)
    assert x.shape == (B, N, N, C), x.shape

    # Filter taps from A (Haar: p = q = 1/sqrt(2)).
    p = float(A[0, 0])
    q = float(A[H, 0])

    key = (p, q)
    if key not in _PROGRAM_CACHE:
        _PROGRAM_CACHE[key] = _build_program(p, q)
    nc = _PROGRAM_CACHE[key]

    in_maps = [{"x": x[b]} for b in range(B)]
    res = run_bass_kernel_spmd(nc, in_maps, core_ids=list(range(B)))
    LAST_RESULTS = res
    return np.stack([res.results[b]["out"] for b in range(B)], axis=0)


# revision 11
# speedup vs baseline: 1.0077x; 1.0077x over previous
"""2D Haar DWT (periodized, 2-tap orthogonal filter bank) on Trainium2.

Reference computes, per batch & channel, y = A @ X @ A^T with A the
2-sparse Haar analysis matrix, then stacks the LL/LH/HL/HH quadrants on
the channel axis.  Because every row of A has exactly two taps
(lowpass p = A[0,0] twice; highpass q = A[H,0], -q), the whole thing is
an elementwise 2x2 butterfly:

    S = E + O     (row pairs: even rows E, odd rows O)
    D = E - O
    LL = p*p*(S_e + S_o)   LH = p*q*(D_e + D_o)
    HL = p*q*(S_e - S_o)   HH = q*q*(D_e - D_o)

which is memory-bound: 16 MiB in + 16 MiB out per core.

Sharding: data-parallel over batch.  Core b gets x[b] (512,512,16) and
produces out[b] (256,256,64).  The two filter taps are read from A on
the host and baked into the program as immediates, so A is never DMA'd.

Engine split per column chunk (DVE is the scarce engine):
  DVE:    S = E+O, and the four quadrant butterflies
  GpSimd: D = E-O (contiguous 2-input op, ~2x DVE cost but POOL is idle)
  ACT:    output scaling (one fused op when p == q, else per-quadrant)
  Sync:   HWDGE DMA
"""

import numpy as np

B, N, C = 8, 512, 16
H = N // 2
P = 128                 # SBUF partitions
COL_CHUNK = 128         # input columns per chunk
FE = COL_CHUNK * C      # free elems of an E/O/S/D tile  (4096)
FV = (COL_CHUNK // 2) * 4 * C  # free elems of a V (output) tile (8192)

_PROGRAM_CACHE = {}


def _build_program(p: float, q: float):
    import concourse.bacc as bacc
    import concourse.mybir as mybir
    from concourse.tile import TileContext

    f32 = mybir.dt.float32
    nc = bacc.Bacc("TRN2", target_bir_lowering=False)

    x = nc.dram_tensor("x", [N, N, C], f32, kind="ExternalInput")
    out = nc.dram_tensor("out", [H, H, 4 * C], f32, kind="ExternalOutput")

    # [2, 256, 8192]: even/odd input rows, flattened (col, chan) free dim
    xr = x[:, :, :].rearrange("(k two) w c -> two k (w c)", two=2)
    # [256, 16384]: output rows, flattened (col, chan) free dim
    of = out[:, :, :].rearrange("k m c -> k (m c)")

    pp, pq, qq = p * p, p * q, q * q
    uniform_scale = abs(p - q) < 1e-12

    with TileContext(nc) as tc:
        with tc.tile_pool(name="pool", bufs=4) as pool:
            for rt in range(H // P):              # 2 row tiles of 128 row-pairs
                r0 = rt * P
                for cc in range(N // COL_CHUNK):  # 4 column chunks
                    f0 = cc * FE
                    e = pool.tile([P, FE], f32)
                    o = pool.tile([P, FE], f32)
                    nc.sync.dma_start(out=e, in_=xr[0, r0 : r0 + P, f0 : f0 + FE])
                    nc.sync.dma_start(out=o, in_=xr[1, r0 : r0 + P, f0 : f0 + FE])

                    s = pool.tile([P, FE], f32, bufs=2)
                    d = pool.tile([P, FE], f32, bufs=2)
                    nc.vector.tensor_add(out=s, in0=e, in1=o)
                    nc.vector.tensor_sub(out=d, in0=e, in1=o)

                    s4 = s.rearrange("p (m two c) -> p m two c", two=2, c=C)
                    d4 = d.rearrange("p (m two c) -> p m two c", two=2, c=C)
                    v = pool.tile([P, FV], f32)
                    v4 = v.rearrange("p (m q c) -> p m q c", q=4, c=C)

                    nc.vector.tensor_add(out=v4[:, :, 0, :], in0=s4[:, :, 0, :], in1=s4[:, :, 1, :])
                    nc.vector.tensor_add(out=v4[:, :, 1, :], in0=d4[:, :, 0, :], in1=d4[:, :, 1, :])
                    nc.vector.tensor_sub(out=v4[:, :, 2, :], in0=s4[:, :, 0, :], in1=s4[:, :, 1, :])
                    nc.vector.tensor_sub(out=v4[:, :, 3, :], in0=d4[:, :, 0, :], in1=d4[:, :, 1, :])

                    if uniform_scale:
                        nc.scalar.mul(v, v, pp)
                    else:
                        nc.scalar.mul(v4[:, :, 0, :], v4[:, :, 0, :], pp)
                        nc.scalar.mul(v4[:, :, 1, :], v4[:, :, 1, :], pq)
                        nc.scalar.mul(v4[:, :, 2, :], v4[:, :, 2, :], pq)
                        nc.scalar.mul(v4[:, :, 3, :], v4[:, :, 3, :], qq)

                    g0 = cc * FV
                    # out-DMA on the scalar engine (HWDGE): it directly
                    # follows its only producer (the ACT scale) in that
                    # engine's stream, and keeps the Sync engine's in-order
                    # stream free for input prefetch.
                    nc.scalar.dma_start(out=of[r0 : r0 + P, g0 : g0 + FV], in_=v)

    nc.finalize()
    return nc


LAST_RESULTS = None  # BassKernelResults of the most recent run (for test harness)


def kernel(x: np.ndarray, A: np.ndarray) -> np.ndarray:
    from concourse.bass_utils import run_bass_kernel_spmd

    global LAST_RESULTS

    x = np.ascontiguousarray(np.asarray(x, dtype=np.float32))
    A = np.asarray(A, dtype=np.float32)
    assert x.shape == (B, N, N, C), x.shape

    # Filter taps from A (Haar: p = q = 1/sqrt(2)).
    p = float(A[0, 0])
    q = float(A[H, 0])

    key = (p, q)
    if key not in _PROGRAM_CACHE:
        _PROGRAM_CACHE[key] = _build_program(p, q)
    nc = _PROGRAM_CACHE[key]

    in_maps = [{"x": x[b]} for b in range(B)]
    res = run_bass_kernel_spmd(nc, in_maps, core_ids=list(range(B)))
    LAST_RESULTS = res
    return np.stack([res.results[b]["out"] for b in range(B)], axis=0)


# revision 12
# speedup vs baseline: 1.1173x; 1.1088x over previous
"""2D Haar DWT (periodized, 2-tap orthogonal filter bank) on Trainium2.

Reference computes, per batch & channel, y = A @ X @ A^T with A the
2-sparse Haar analysis matrix, then stacks the LL/LH/HL/HH quadrants on
the channel axis.  Because every row of A has exactly two taps
(lowpass p = A[0,0] twice; highpass q = A[H,0], -q), the whole thing is
an elementwise 2x2 butterfly:

    S = E + O     (row pairs: even rows E, odd rows O)
    D = E - O
    LL = p*p*(S_e + S_o)   LH = p*q*(D_e + D_o)
    HL = p*q*(S_e - S_o)   HH = q*q*(D_e - D_o)

which is memory-bound: 16 MiB in + 16 MiB out per core.

Sharding: data-parallel over batch.  Core b gets x[b] (512,512,16) and
produces out[b] (256,256,64).  The two filter taps are read from A on
the host and baked into the program as immediates, so A is never DMA'd.

Engine split per column chunk (DVE is the scarce engine):
  DVE:    S = E+O, and the four quadrant butterflies
  GpSimd: D = E-O (contiguous 2-input op, ~2x DVE cost but POOL is idle)
  ACT:    output scaling (one fused op when p == q, else per-quadrant)
  Sync:   HWDGE DMA
"""

import numpy as np

B, N, C = 8, 512, 16
H = N // 2
P = 128                 # SBUF partitions
COL_CHUNK = 128         # input columns per chunk
FE = COL_CHUNK * C      # free elems of an E/O/S/D tile  (4096)
FV = (COL_CHUNK // 2) * 4 * C  # free elems of a V (output) tile (8192)

_PROGRAM_CACHE = {}


def _build_program(p: float, q: float):
    import concourse.bacc as bacc
    import concourse.mybir as mybir
    from concourse.tile import TileContext

    f32 = mybir.dt.float32
    nc = bacc.Bacc("TRN2", target_bir_lowering=False)

    x = nc.dram_tensor("x", [N, N, C], f32, kind="ExternalInput")
    out = nc.dram_tensor("out", [H, H, 4 * C], f32, kind="ExternalOutput")

    # [2, 256, 8192]: even/odd input rows, flattened (col, chan) free dim
    xr = x[:, :, :].rearrange("(k two) w c -> two k (w c)", two=2)
    # [256, 16384]: output rows, flattened (col, chan) free dim
    of = out[:, :, :].rearrange("k m c -> k (m c)")

    pp, pq, qq = p * p, p * q, q * q
    uniform_scale = abs(p - q) < 1e-12

    with TileContext(nc) as tc:
        with tc.tile_pool(name="pool", bufs=4) as pool:
            for rt in range(H // P):              # 2 row tiles of 128 row-pairs
                r0 = rt * P
                for cc in range(N // COL_CHUNK):  # 4 column chunks
                    f0 = cc * FE
                    e = pool.tile([P, FE], f32)
                    o = pool.tile([P, FE], f32)
                    nc.sync.dma_start(out=e, in_=xr[0, r0 : r0 + P, f0 : f0 + FE])
                    nc.sync.dma_start(out=o, in_=xr[1, r0 : r0 + P, f0 : f0 + FE])

                    s = pool.tile([P, FE], f32, bufs=2)
                    d = pool.tile([P, FE], f32, bufs=2)
                    nc.vector.tensor_add(out=s, in0=e, in1=o)
                    nc.vector.tensor_sub(out=d, in0=e, in1=o)

                    s4 = s.rearrange("p (m two c) -> p m two c", two=2, c=C)
                    d4 = d.rearrange("p (m two c) -> p m two c", two=2, c=C)
                    v = pool.tile([P, FV], f32)
                    v4 = v.rearrange("p (m q c) -> p m q c", q=4, c=C)

                    nc.vector.tensor_add(out=v4[:, :, 0, :], in0=s4[:, :, 0, :], in1=s4[:, :, 1, :])
                    nc.vector.tensor_add(out=v4[:, :, 1, :], in0=d4[:, :, 0, :], in1=d4[:, :, 1, :])
                    nc.vector.tensor_sub(out=v4[:, :, 2, :], in0=s4[:, :, 0, :], in1=s4[:, :, 1, :])
                    nc.vector.tensor_sub(out=v4[:, :, 3, :], in0=d4[:, :, 0, :], in1=d4[:, :, 1, :])

                    if uniform_scale:
                        nc.scalar.mul(v, v, pp)
                    else:
                        nc.scalar.mul(v4[:, :, 0, :], v4[:, :, 0, :], pp)
                        nc.scalar.mul(v4[:, :, 1, :], v4[:, :, 1, :], pq)
                        nc.scalar.mul(v4[:, :, 2, :], v4[:, :, 2, :], pq)
                        nc.scalar.mul(v4[:, :, 3, :], v4[:, :, 3, :], qq)

                    g0 = cc * FV
                    # out-DMA on the scalar engine (HWDGE): it directly
                    # follows its only producer (the ACT scale) in that
                    # engine's stream, and keeps the Sync engine's in-order
                    # stream free for input prefetch.
                    nc.scalar.dma_start(out=of[r0 : r0 + P, g0 : g0 + FV], in_=v)

    nc.finalize()
    return nc


LAST_RESULTS = None  # BassKernelResults of the most recent run (for test harness)


def _ensure_axon_hooks_importable():
    """bass_utils imports antenv.axon_hooks when BASS_TRACE is set; some
    images lack that module, which would turn a stray BASS_TRACE=1 into a
    crash.  Install a stub whose hook getter returns None (bass_utils then
    skips tracing gracefully).  A real hook installed earlier wins."""
    import sys
    import types

    try:
        import antenv.axon_hooks  # noqa: F401
    except ImportError:
        mod = types.ModuleType("antenv.axon_hooks")
        mod.get_axon_ntff_profile_hook = lambda: None
        mod.set_axon_ntff_profile_hook = lambda h: None
        sys.modules["antenv.axon_hooks"] = mod
        try:
            import antenv

            antenv.axon_hooks = mod
        except ImportError:
            pass


def kernel(x: np.ndarray, A: np.ndarray) -> np.ndarray:
    _ensure_axon_hooks_importable()
    from concourse.bass_utils import run_bass_kernel_spmd

    global LAST_RESULTS

    x = np.ascontiguousarray(np.asarray(x, dtype=np.float32))
    A = np.asarray(A, dtype=np.float32)
    assert x.shape == (B, N, N, C), x.shape

    # Filter taps from A (Haar: p = q = 1/sqrt(2)).
    p = float(A[0, 0])
    q = float(A[H, 0])

    key = (p, q)
    if key not in _PROGRAM_CACHE:
        _PROGRAM_CACHE[key] = _build_program(p, q)
    nc = _PROGRAM_CACHE[key]

    in_maps = [{"x": x[b]} for b in range(B)]
    res = run_bass_kernel_spmd(nc, in_maps, core_ids=list(range(B)))
    LAST_RESULTS = res
    return np.stack([res.results[b]["out"] for b in range(B)], axis=0)
